# revision 7
# baseline (speedup 1.0000x reference)
"""Multi-head causal attention kernel for Trainium2 (8 NeuronCores, batch-parallel).

Problem: B=8, Tx=Tz=1024, Dx=Dz=1024, Datt=Dmid=64, H=16, Dout=1024, causal mask.
Sharding: batch dim across the 8 cores (one batch element per core) - weights
replicated, no collectives needed.

Per-core dataflow (matmul operands in fp16, all accumulation in fp32 PSUM):
  xT,zT : host-transposed activations [d, t]
  V     = zT.T @ Wv_cat + bv    -> [z, h*65+e] with a ones column per head
                                   (bias via K=1 matmul; ones col via DVE fill)
  per head-pair P (2 heads at partition offsets 0/64):
    QT_P = Wq_cat[:,P].T @ xT + bq  -> [128(he), 1024(x)]   (evict DVE+bias)
    KT_P = Wk_cat[:,P].T @ zT + bk  -> [128(he), 1024(z)]
    S^T  = lhsT=KT[64,128] x rhs=QT[64,512] -> 2-bank psum [z, 2*x] (row-packed)
    A^T  = exp(S^T/8) both heads in one ACT op, causal-trimmed, diag masked (DVE)
    yT   = V_aug.T @ A^T -> psum [65, x]: rows 0..63 = y^T, row 64 = sumexp
    norm: 1/sumexp (DVE approx) -> partition_broadcast (GPSIMD) -> mul (DVE)
  out  = yT_cat.T @ Wp + bp  (psum -> SBUF via ACT -> DRAM)
"""
import sys

sys.path.insert(0, "/opt/trn_rl_repo")

import numpy as np

import concourse.bacc as bacc
import concourse.mybir as mybir
import concourse.tile as tile
from concourse.bass_utils import run_bass_kernel_spmd

F32 = mybir.dt.float32
FP16 = mybir.dt.float16

B, T, D, E, H = 8, 1024, 1024, 64, 16
NK = D // 128          # 8 contraction tiles
NP = H // 2            # 8 head pairs
NJ = T // 128          # 8 z tiles
NC = T // 512          # 2 x chunks
SCALE = 0.125          # 1/sqrt(64)


def build_program():
    nc = bacc.Bacc("TRN2", target_bir_lowering=False, debug=False)

    xT_d = nc.dram_tensor("xT", [D, T], FP16, kind="ExternalInput")
    zT_d = nc.dram_tensor("zT", [D, T], FP16, kind="ExternalInput")
    wq_d = nc.dram_tensor("wq", [D, H * E], FP16, kind="ExternalInput")
    wk_d = nc.dram_tensor("wk", [D, H * E], FP16, kind="ExternalInput")
    wv_d = nc.dram_tensor("wv", [D, H * E], FP16, kind="ExternalInput")
    wp_d = nc.dram_tensor("wp", [H * E, D], FP16, kind="ExternalInput")
    bqk_d = nc.dram_tensor("bqk", [128, 16], F32, kind="ExternalInput")
    bvp_d = nc.dram_tensor("bvp", [65, H * E], FP16, kind="ExternalInput")
    maskt_d = nc.dram_tensor("maskt", [128, 256], FP16, kind="ExternalInput")
    ones128_d = nc.dram_tensor("ones128", [65, 128], FP16, kind="ExternalInput")
    out_d = nc.dram_tensor("out", [T, D], F32, kind="ExternalOutput")

    Exp = mybir.ActivationFunctionType.Exp

    with tile.TileContext(nc) as tc:
        with (
            tc.tile_pool(name="big", bufs=1) as big,
            tc.tile_pool(name="wf", bufs=2) as wf,
            tc.tile_pool(name="wb", bufs=4) as wb,
            tc.tile_pool(name="qk", bufs=4) as qk,
            tc.tile_pool(name="apool", bufs=8) as apool,
            tc.tile_pool(name="norm", bufs=3) as norm,
            tc.tile_pool(name="opool", bufs=3) as opool,
            tc.tile_pool(name="cst", bufs=1) as cst,
            tc.tile_pool(name="mps", bufs=2, space="PSUM") as mps,
            tc.tile_pool(name="sps", bufs=2, space="PSUM") as sps,
            tc.tile_pool(name="yps", bufs=2, space="PSUM") as yps,
        ):
            # ---- constants ----
            bqk_t = cst.tile([128, 16], F32)
            bvp_t = cst.tile([65, H * E], FP16)
            maskt_t = cst.tile([128, 256], FP16)
            ones128_t = cst.tile([65, 128], FP16)
            onesf_t = cst.tile([128, 16], FP16)
            nc.gpsimd.dma_start(bqk_t[:], bqk_d.ap())
            nc.gpsimd.dma_start(bvp_t[:], bvp_d.ap())
            nc.gpsimd.dma_start(maskt_t[:], maskt_d.ap())
            nc.gpsimd.dma_start(ones128_t[:], ones128_d.ap())
            nc.gpsimd.memset(onesf_t[:], 1.0)

            # ---- resident activations ----
            xT_t = big.tile([128, NK, T], FP16, tag="xT")
            zT_t = big.tile([128, NK, T], FP16, tag="zT")
            V_t = big.tile([128, NJ, H * 65], FP16, tag="V")
            yT_t = big.tile([128, NP, T], FP16, tag="yT")
            wv_r = wv_d.ap().rearrange("(k p) he -> p k he", p=128)
            wp_r = wp_d.ap().rearrange("(k p) dout -> p k dout", p=128)
            wq_r = wq_d.ap().rearrange("(k p) he -> p k he", p=128)
            wk_r = wk_d.ap().rearrange("(k p) he -> p k he", p=128)
            xT_r = xT_d.ap().rearrange("(k p) t -> p k t", p=128)
            zT_r = zT_d.ap().rearrange("(k p) t -> p k t", p=128)
            # zT + first wv half first: V-phase matmuls can start after k=0 lands
            wvh0 = wf.tile([128, NK, 512], FP16, tag="wf", name="wvh0")
            for k in range(NK):
                nc.sync.dma_start(zT_t[:, k, :], zT_r[:, k, :])
                nc.sync.dma_start(wvh0[:, k, :], wv_r[:, k, 0:512])
            for k in range(NK):
                nc.gpsimd.dma_start(xT_t[:, k, :], xT_r[:, k, :])

            # ---- V phase: V[z, he] = zT.T @ Wv + bv (65-col/head layout) ----
            for vc in range(2):
                if vc == 0:
                    wvh = wvh0
                else:
                    wvh = wf.tile([128, NK, 512], FP16, tag="wf")
                    nc.sync.dma_start(wvh[:], wv_r[:, :, vc * 512:(vc + 1) * 512])
                for zb in range(NJ):
                    ps = mps.tile([128, 512], F32, tag="mps")
                    for k in range(NK):
                        nc.tensor.matmul(
                            ps[:], zT_t[:, k, zb * 128:(zb + 1) * 128], wvh[:, k, :],
                            start=(k == 0), stop=False,
                        )
                    nc.tensor.matmul(
                        ps[:], ones128_t[0:1, :], bvp_t[0:1, vc * 512:(vc + 1) * 512],
                        start=False, stop=True,
                    )
                    dst = V_t[:, zb, vc * 520:(vc + 1) * 520].rearrange(
                        "p (h c) -> p h c", c=65)[:, :, 0:64]
                    nc.scalar.copy(dst, ps[:].rearrange("p (h c) -> p h c", c=64))
            for zb in range(NJ):
                ones_dst = V_t[:, zb, :].rearrange("p (h c) -> p h c", c=65)[:, :, 64:65]
                nc.vector.tensor_copy(ones_dst, onesf_t[:].rearrange("p (h c) -> p h c", c=1))

            # ---- prefetch both Wp halves (wf slots free up after V-phase use) ----
            wph = []
            for dc in range(2):
                w = wf.tile([128, NK, 512], FP16, tag="wf", name=f"wph{dc}")
                nc.sync.dma_start(w[:], wp_r[:, :, dc * 512:(dc + 1) * 512])
                wph.append(w)

            # ---- head-pair loop ----
            for P in range(NP):
                wqP = wb.tile([128, NK, 128], FP16, tag="wb")
                nc.sync.dma_start(wqP[:], wq_r[:, :, P * 128:(P + 1) * 128])
                wkP = wb.tile([128, NK, 128], FP16, tag="wb")
                nc.sync.dma_start(wkP[:], wk_r[:, :, P * 128:(P + 1) * 128])

                QT = qk.tile([128, T], FP16, tag="qk")
                for c in range(NC):
                    ps = mps.tile([128, 512], F32, tag="mps")
                    for k in range(NK):
                        nc.tensor.matmul(
                            ps[:], wqP[:, k, :], xT_t[:, k, c * 512:(c + 1) * 512],
                            start=(k == 0), stop=(k == NK - 1),
                        )
                    nc.vector.tensor_scalar_add(QT[:, c * 512:(c + 1) * 512], ps[:],
                                                bqk_t[:, P:P + 1])
                KT = qk.tile([128, T], FP16, tag="qk")
                for c in range(NC):
                    ps = mps.tile([128, 512], F32, tag="mps")
                    for k in range(NK):
                        nc.tensor.matmul(
                            ps[:], wkP[:, k, :], zT_t[:, k, c * 512:(c + 1) * 512],
                            start=(k == 0), stop=(k == NK - 1),
                        )
                    nc.vector.tensor_scalar_add(KT[:, c * 512:(c + 1) * 512], ps[:],
                                                bqk_t[:, 8 + P:9 + P])

                # attention for the two heads of this pair
                for c in range(NC):
                    jlive = [j for j in range(NJ) if 128 * j <= 512 * c + 511]
                    yp = [yps.tile([65, 512], F32, tag="yps", name=f"yp{P}_{c}_{h01}")
                          for h01 in range(2)]
                    for j in jlive:
                        kband = j - 4 * c
                        x0 = 128 * max(kband, 0)
                        sp = sps.tile([128, 1024], F32, tag="sps")
                        at = apool.tile([128, 1024], FP16, tag="at")
                        for h01 in range(2):
                            hoff = 64 * h01
                            nc.tensor.matmul(
                                sp[:, h01 * 512 + x0:(h01 + 1) * 512],
                                KT[hoff:hoff + 64, j * 128:(j + 1) * 128],
                                QT[hoff:hoff + 64, c * 512 + x0:(c + 1) * 512],
                                start=True, stop=True,
                            )
                        # one exp over both heads' regions (strided 2-bank AP)
                        sp_v = sp[:].rearrange("p (h x) -> p h x", x=512)[:, :, x0:512]
                        at_v = at[:].rearrange("p (h x) -> p h x", x=512)[:, :, x0:512]
                        nc.scalar.activation(at_v, sp_v, Exp, bias=0.0, scale=SCALE)
                        if kband >= 0:
                            at_m = at[:].rearrange(
                                "p (h x) -> p h x", x=512)[:, :, x0:x0 + 128]
                            mk_m = maskt_t[:].rearrange("p (h x) -> p h x", x=128)
                            nc.vector.tensor_mul(at_m, at_m, mk_m)
                        for h01 in range(2):
                            h = 2 * P + h01
                            nc.tensor.matmul(
                                yp[h01][:, x0:512],
                                V_t[:, j, h * 65:(h + 1) * 65],
                                at[:, h01 * 512 + x0:(h01 + 1) * 512],
                                start=(j == jlive[0]), stop=(j == jlive[-1]),
                                skip_group_check=True,
                            )
                    # normalization + eviction to packed pair layout
                    for h01 in range(2):
                        hoff = 64 * h01
                        se_t = norm.tile([1, 512], F32, tag="se")
                        nc.vector.tensor_copy(se_t[:], yp[h01][64:65, :])
                        r_t = norm.tile([1, 512], F32, tag="rt")
                        nc.vector.reciprocal_approx_fast(r_t[:], se_t[:])
                        bc_t = norm.tile([64, 512], F32, tag="bc")
                        nc.gpsimd.partition_broadcast(bc_t[:], r_t[:])
                        nc.vector.tensor_mul(
                            yT_t[hoff:hoff + 64, P, c * 512:(c + 1) * 512],
                            yp[h01][0:64, :], bc_t[:])

            # ---- output projection: out = yT_cat.T @ Wp + bp ----
            for dc in range(2):
                for m in range(NJ):
                    ps = mps.tile([128, 512], F32, tag="mps")
                    for ht in range(NP):
                        nc.tensor.matmul(
                            ps[:], yT_t[:, ht, m * 128:(m + 1) * 128], wph[dc][:, ht, :],
                            start=(ht == 0), stop=False,
                        )
                    nc.tensor.matmul(
                        ps[:], ones128_t[64:65, :], bvp_t[64:65, dc * 512:(dc + 1) * 512],
                        start=False, stop=True,
                    )
                    o_t = opool.tile([128, 512], F32, tag="ot")
                    nc.vector.tensor_copy(o_t[:], ps[:])
                    nc.sync.dma_start(
                        out_d.ap()[m * 128:(m + 1) * 128, dc * 512:(dc + 1) * 512],
                        o_t[:])

    nc.compile()
    return nc


_CACHED_NC = None


def _get_program():
    global _CACHED_NC
    if _CACHED_NC is None:
        _CACHED_NC = build_program()
    return _CACHED_NC


def _ones65x128():
    a = np.zeros((65, 128), np.float16)
    a[0] = 1.0
    a[64] = 1.0
    return a


def _prep_shared(Wq, bq, Wk, bk, Wv, bv, Wp, bp, mask):
    assert np.array_equal(
        np.asarray(mask), np.tril(np.ones((T, T), dtype=bool))
    ), "kernel specialized for causal (tril) mask"
    wq = np.ascontiguousarray(
        np.asarray(Wq, np.float32).transpose(1, 0, 2).reshape(D, H * E).astype(np.float16))
    wk = np.ascontiguousarray(
        np.asarray(Wk, np.float32).transpose(1, 0, 2).reshape(D, H * E).astype(np.float16))
    wv = np.ascontiguousarray(
        np.asarray(Wv, np.float32).transpose(1, 0, 2).reshape(D, H * E).astype(np.float16))
    wp = np.ascontiguousarray(np.asarray(Wp, np.float32).astype(np.float16))
    bq_c = np.asarray(bq, np.float32).reshape(-1)
    bk_c = np.asarray(bk, np.float32).reshape(-1)
    bqk = np.concatenate(
        [bq_c.reshape(8, 128).T, bk_c.reshape(8, 128).T], axis=1
    ).astype(np.float32)
    bvp = np.zeros((65, H * E), np.float16)
    bvp[0] = np.asarray(bv, np.float32).reshape(-1).astype(np.float16)
    bvp[64] = np.asarray(bp, np.float32).reshape(-1).astype(np.float16)
    tri = np.triu(np.ones((128, 128), np.float16))  # allow z <= x
    maskt = np.concatenate([tri, tri], axis=1)      # [128, 256] for both heads
    return {
        "wq": wq, "wk": wk, "wv": wv, "wp": wp,
        "bqk": np.ascontiguousarray(bqk), "bvp": np.ascontiguousarray(bvp),
        "maskt": np.ascontiguousarray(maskt),
        "ones128": _ones65x128(),
    }


def kernel(x, z, Wq, bq, Wk, bk, Wv, bv, Wp, bp, mask, _trace=False, _trace_kwargs=None):
    x = np.asarray(x, np.float32)
    z = np.asarray(z, np.float32)
    shared = _prep_shared(Wq, bq, Wk, bk, Wv, bv, Wp, bp, mask)
    in_maps = []
    for b in range(B):
        m = dict(shared)
        m["xT"] = np.ascontiguousarray(x[b].T.astype(np.float16))
        m["zT"] = np.ascontiguousarray(z[b].T.astype(np.float16))
        in_maps.append(m)
    nc = _get_program()
    res = run_bass_kernel_spmd(
        nc, in_maps, core_ids=list(range(B)),
        trace=_trace, **(_trace_kwargs or {}),
    )
    out = np.stack([r["out"] for r in res.results]).astype(np.float32)
    if _trace:
        kernel.last_results = res
    return out


# revision 8
# speedup vs baseline: 1.0312x; 1.0312x over previous
"""Multi-head causal attention kernel for Trainium2 (8 NeuronCores, batch-parallel).

Problem: B=8, Tx=Tz=1024, Dx=Dz=1024, Datt=Dmid=64, H=16, Dout=1024, causal mask.
Sharding: batch dim across the 8 cores (one batch element per core) - weights
replicated, no collectives needed.

Per-core dataflow (matmul operands in fp16, all accumulation in fp32 PSUM):
  xT,zT : host-transposed activations [d, t]
  V     = zT.T @ Wv_cat + bv    -> [z, h*65+e] with a ones column per head
                                   (bias via K=1 matmul; ones col via DVE fill)
  per head-pair P (2 heads at partition offsets 0/64):
    QT_P = Wq_cat[:,P].T @ xT + bq  -> [128(he), 1024(x)]   (evict DVE+bias)
    KT_P = Wk_cat[:,P].T @ zT + bk  -> [128(he), 1024(z)]
    S^T  = lhsT=KT[64,128] x rhs=QT[64,512] -> 2-bank psum [z, 2*x] (row-packed)
    A^T  = exp(S^T/8) both heads in one ACT op, causal-trimmed, diag masked (DVE)
    yT   = V_aug.T @ A^T -> psum [65, x]: rows 0..63 = y^T, row 64 = sumexp
    norm: 1/sumexp (DVE approx) -> partition_broadcast (GPSIMD) -> mul (DVE)
  out  = yT_cat.T @ Wp + bp  (psum -> SBUF via ACT -> DRAM)
"""
import sys

sys.path.insert(0, "/opt/trn_rl_repo")

import numpy as np

import concourse.bacc as bacc
import concourse.mybir as mybir
import concourse.tile as tile
from concourse.bass_utils import run_bass_kernel_spmd

F32 = mybir.dt.float32
FP16 = mybir.dt.float16

B, T, D, E, H = 8, 1024, 1024, 64, 16
NK = D // 128          # 8 contraction tiles
NP = H // 2            # 8 head pairs
NJ = T // 128          # 8 z tiles
NC = T // 512          # 2 x chunks
SCALE = 0.125          # 1/sqrt(64)


def build_program():
    nc = bacc.Bacc("TRN2", target_bir_lowering=False, debug=False)

    xT_d = nc.dram_tensor("xT", [D, T], FP16, kind="ExternalInput")
    zT_d = nc.dram_tensor("zT", [D, T], FP16, kind="ExternalInput")
    wq_d = nc.dram_tensor("wq", [D, H * E], FP16, kind="ExternalInput")
    wk_d = nc.dram_tensor("wk", [D, H * E], FP16, kind="ExternalInput")
    wv_d = nc.dram_tensor("wv", [D, H * E], FP16, kind="ExternalInput")
    wp_d = nc.dram_tensor("wp", [H * E, D], FP16, kind="ExternalInput")
    bqk_d = nc.dram_tensor("bqk", [128, 16], F32, kind="ExternalInput")
    bvp_d = nc.dram_tensor("bvp", [65, H * E], FP16, kind="ExternalInput")
    maskt_d = nc.dram_tensor("maskt", [128, 256], FP16, kind="ExternalInput")
    ones128_d = nc.dram_tensor("ones128", [65, 128], FP16, kind="ExternalInput")
    out_d = nc.dram_tensor("out", [T, D], F32, kind="ExternalOutput")

    Exp = mybir.ActivationFunctionType.Exp

    with tile.TileContext(nc) as tc:
        with (
            tc.tile_pool(name="big", bufs=1) as big,
            tc.tile_pool(name="wf", bufs=2) as wf,
            tc.tile_pool(name="wb", bufs=4) as wb,
            tc.tile_pool(name="qk", bufs=4) as qk,
            tc.tile_pool(name="apool", bufs=8) as apool,
            tc.tile_pool(name="norm", bufs=3) as norm,
            tc.tile_pool(name="opool", bufs=3) as opool,
            tc.tile_pool(name="cst", bufs=1) as cst,
            tc.tile_pool(name="mps", bufs=2, space="PSUM") as mps,
            tc.tile_pool(name="sps", bufs=2, space="PSUM") as sps,
            tc.tile_pool(name="yps", bufs=2, space="PSUM") as yps,
        ):
            # ---- constants ----
            bqk_t = cst.tile([128, 16], F32)
            bvp_t = cst.tile([65, H * E], FP16)
            maskt_t = cst.tile([128, 256], FP16)
            ones128_t = cst.tile([65, 128], FP16)
            onesf_t = cst.tile([128, 16], FP16)
            def _load_consts():
                nc.sync.dma_start(bqk_t[:], bqk_d.ap())
                nc.sync.dma_start(bvp_t[:], bvp_d.ap())
                nc.sync.dma_start(maskt_t[:], maskt_d.ap())
                nc.sync.dma_start(ones128_t[:], ones128_d.ap())
            nc.gpsimd.memset(onesf_t[:], 1.0)

            # ---- resident activations ----
            xT_t = big.tile([128, NK, T], FP16, tag="xT")
            zT_t = big.tile([128, NK, T], FP16, tag="zT")
            V_t = big.tile([128, NJ, H * 65], FP16, tag="V")
            yT_t = big.tile([128, NP, T], FP16, tag="yT")
            wv_r = wv_d.ap().rearrange("(k p) he -> p k he", p=128)
            wp_r = wp_d.ap().rearrange("(k p) dout -> p k dout", p=128)
            wq_r = wq_d.ap().rearrange("(k p) he -> p k he", p=128)
            wk_r = wk_d.ap().rearrange("(k p) he -> p k he", p=128)
            xT_r = xT_d.ap().rearrange("(k p) t -> p k t", p=128)
            zT_r = zT_d.ap().rearrange("(k p) t -> p k t", p=128)
            # zT + first wv half first: V-phase matmuls can start after k=0 lands
            wvh0 = wf.tile([128, NK, 512], FP16, tag="wf", name="wvh0")
            for k in range(NK):
                nc.sync.dma_start(zT_t[:, k, :], zT_r[:, k, :])
                nc.sync.dma_start(wvh0[:, k, :], wv_r[:, k, 0:512])
            _load_consts()
            for k in range(NK):
                nc.sync.dma_start(xT_t[:, k, :], xT_r[:, k, :])

            # ---- V phase: V[z, he] = zT.T @ Wv + bv (65-col/head layout) ----
            for vc in range(2):
                if vc == 0:
                    wvh = wvh0
                else:
                    wvh = wf.tile([128, NK, 512], FP16, tag="wf")
                    nc.sync.dma_start(wvh[:], wv_r[:, :, vc * 512:(vc + 1) * 512])
                for zb in range(NJ):
                    ps = mps.tile([128, 512], F32, tag="mps")
                    for k in range(NK):
                        nc.tensor.matmul(
                            ps[:], zT_t[:, k, zb * 128:(zb + 1) * 128], wvh[:, k, :],
                            start=(k == 0), stop=False,
                        )
                    nc.tensor.matmul(
                        ps[:], ones128_t[0:1, :], bvp_t[0:1, vc * 512:(vc + 1) * 512],
                        start=False, stop=True,
                    )
                    dst = V_t[:, zb, vc * 520:(vc + 1) * 520].rearrange(
                        "p (h c) -> p h c", c=65)[:, :, 0:64]
                    nc.scalar.copy(dst, ps[:].rearrange("p (h c) -> p h c", c=64))
            for zb in range(NJ):
                ones_dst = V_t[:, zb, :].rearrange("p (h c) -> p h c", c=65)[:, :, 64:65]
                nc.vector.tensor_copy(ones_dst, onesf_t[:].rearrange("p (h c) -> p h c", c=1))

            # ---- prefetch both Wp halves (wf slots free up after V-phase use) ----
            wph = []
            for dc in range(2):
                w = wf.tile([128, NK, 512], FP16, tag="wf", name=f"wph{dc}")
                nc.sync.dma_start(w[:], wp_r[:, :, dc * 512:(dc + 1) * 512])
                wph.append(w)

            # ---- head-pair loop ----
            for P in range(NP):
                wqP = wb.tile([128, NK, 128], FP16, tag="wb")
                nc.sync.dma_start(wqP[:], wq_r[:, :, P * 128:(P + 1) * 128])
                wkP = wb.tile([128, NK, 128], FP16, tag="wb")
                nc.sync.dma_start(wkP[:], wk_r[:, :, P * 128:(P + 1) * 128])

                QT = qk.tile([128, T], FP16, tag="qk")
                for c in range(NC):
                    ps = mps.tile([128, 512], F32, tag="mps")
                    for k in range(NK):
                        nc.tensor.matmul(
                            ps[:], wqP[:, k, :], xT_t[:, k, c * 512:(c + 1) * 512],
                            start=(k == 0), stop=(k == NK - 1),
                        )
                    nc.vector.tensor_scalar_add(QT[:, c * 512:(c + 1) * 512], ps[:],
                                                bqk_t[:, P:P + 1])
                KT = qk.tile([128, T], FP16, tag="qk")
                for c in range(NC):
                    ps = mps.tile([128, 512], F32, tag="mps")
                    for k in range(NK):
                        nc.tensor.matmul(
                            ps[:], wkP[:, k, :], zT_t[:, k, c * 512:(c + 1) * 512],
                            start=(k == 0), stop=(k == NK - 1),
                        )
                    nc.vector.tensor_scalar_add(KT[:, c * 512:(c + 1) * 512], ps[:],
                                                bqk_t[:, 8 + P:9 + P])

                # attention for the two heads of this pair
                for c in range(NC):
                    jlive = [j for j in range(NJ) if 128 * j <= 512 * c + 511]
                    yp = [yps.tile([65, 512], F32, tag="yps", name=f"yp{P}_{c}_{h01}")
                          for h01 in range(2)]
                    for j in jlive:
                        kband = j - 4 * c
                        x0 = 128 * max(kband, 0)
                        sp = sps.tile([128, 1024], F32, tag="sps")
                        at = apool.tile([128, 1024], FP16, tag="at")
                        for h01 in range(2):
                            hoff = 64 * h01
                            nc.tensor.matmul(
                                sp[:, h01 * 512 + x0:(h01 + 1) * 512],
                                KT[hoff:hoff + 64, j * 128:(j + 1) * 128],
                                QT[hoff:hoff + 64, c * 512 + x0:(c + 1) * 512],
                                start=True, stop=True,
                            )
                        # one exp over both heads' regions (strided 2-bank AP)
                        sp_v = sp[:].rearrange("p (h x) -> p h x", x=512)[:, :, x0:512]
                        at_v = at[:].rearrange("p (h x) -> p h x", x=512)[:, :, x0:512]
                        nc.scalar.activation(at_v, sp_v, Exp, bias=0.0, scale=SCALE)
                        if kband >= 0:
                            at_m = at[:].rearrange(
                                "p (h x) -> p h x", x=512)[:, :, x0:x0 + 128]
                            mk_m = maskt_t[:].rearrange("p (h x) -> p h x", x=128)
                            nc.vector.tensor_mul(at_m, at_m, mk_m)
                        for h01 in range(2):
                            h = 2 * P + h01
                            nc.tensor.matmul(
                                yp[h01][:, x0:512],
                                V_t[:, j, h * 65:(h + 1) * 65],
                                at[:, h01 * 512 + x0:(h01 + 1) * 512],
                                start=(j == jlive[0]), stop=(j == jlive[-1]),
                                skip_group_check=True,
                            )
                    # normalization + eviction to packed pair layout
                    for h01 in range(2):
                        hoff = 64 * h01
                        se_t = norm.tile([1, 512], F32, tag="se")
                        nc.vector.tensor_copy(se_t[:], yp[h01][64:65, :])
                        r_t = norm.tile([1, 512], F32, tag="rt")
                        nc.vector.reciprocal_approx_fast(r_t[:], se_t[:])
                        bc_t = norm.tile([64, 512], F32, tag="bc")
                        nc.gpsimd.partition_broadcast(bc_t[:], r_t[:])
                        nc.vector.tensor_mul(
                            yT_t[hoff:hoff + 64, P, c * 512:(c + 1) * 512],
                            yp[h01][0:64, :], bc_t[:])

            # ---- output projection: out = yT_cat.T @ Wp + bp ----
            for dc in range(2):
                for m in range(NJ):
                    ps = mps.tile([128, 512], F32, tag="mps")
                    for ht in range(NP):
                        nc.tensor.matmul(
                            ps[:], yT_t[:, ht, m * 128:(m + 1) * 128], wph[dc][:, ht, :],
                            start=(ht == 0), stop=False,
                        )
                    nc.tensor.matmul(
                        ps[:], ones128_t[64:65, :], bvp_t[64:65, dc * 512:(dc + 1) * 512],
                        start=False, stop=True,
                    )
                    o_t = opool.tile([128, 512], F32, tag="ot")
                    nc.vector.tensor_copy(o_t[:], ps[:])
                    nc.sync.dma_start(
                        out_d.ap()[m * 128:(m + 1) * 128, dc * 512:(dc + 1) * 512],
                        o_t[:])

    nc.compile()
    return nc


_CACHED_NC = None


def _get_program():
    global _CACHED_NC
    if _CACHED_NC is None:
        _CACHED_NC = build_program()
    return _CACHED_NC


def _ones65x128():
    a = np.zeros((65, 128), np.float16)
    a[0] = 1.0
    a[64] = 1.0
    return a


def _prep_shared(Wq, bq, Wk, bk, Wv, bv, Wp, bp, mask):
    assert np.array_equal(
        np.asarray(mask), np.tril(np.ones((T, T), dtype=bool))
    ), "kernel specialized for causal (tril) mask"
    wq = np.ascontiguousarray(
        np.asarray(Wq, np.float32).transpose(1, 0, 2).reshape(D, H * E).astype(np.float16))
    wk = np.ascontiguousarray(
        np.asarray(Wk, np.float32).transpose(1, 0, 2).reshape(D, H * E).astype(np.float16))
    wv = np.ascontiguousarray(
        np.asarray(Wv, np.float32).transpose(1, 0, 2).reshape(D, H * E).astype(np.float16))
    wp = np.ascontiguousarray(np.asarray(Wp, np.float32).astype(np.float16))
    bq_c = np.asarray(bq, np.float32).reshape(-1)
    bk_c = np.asarray(bk, np.float32).reshape(-1)
    bqk = np.concatenate(
        [bq_c.reshape(8, 128).T, bk_c.reshape(8, 128).T], axis=1
    ).astype(np.float32)
    bvp = np.zeros((65, H * E), np.float16)
    bvp[0] = np.asarray(bv, np.float32).reshape(-1).astype(np.float16)
    bvp[64] = np.asarray(bp, np.float32).reshape(-1).astype(np.float16)
    tri = np.triu(np.ones((128, 128), np.float16))  # allow z <= x
    maskt = np.concatenate([tri, tri], axis=1)      # [128, 256] for both heads
    return {
        "wq": wq, "wk": wk, "wv": wv, "wp": wp,
        "bqk": np.ascontiguousarray(bqk), "bvp": np.ascontiguousarray(bvp),
        "maskt": np.ascontiguousarray(maskt),
        "ones128": _ones65x128(),
    }


def kernel(x, z, Wq, bq, Wk, bk, Wv, bv, Wp, bp, mask, _trace=False, _trace_kwargs=None):
    x = np.asarray(x, np.float32)
    z = np.asarray(z, np.float32)
    shared = _prep_shared(Wq, bq, Wk, bk, Wv, bv, Wp, bp, mask)
    in_maps = []
    for b in range(B):
        m = dict(shared)
        m["xT"] = np.ascontiguousarray(x[b].T.astype(np.float16))
        m["zT"] = np.ascontiguousarray(z[b].T.astype(np.float16))
        in_maps.append(m)
    nc = _get_program()
    res = run_bass_kernel_spmd(
        nc, in_maps, core_ids=list(range(B)),
        trace=_trace, **(_trace_kwargs or {}),
    )
    out = np.stack([r["out"] for r in res.results]).astype(np.float32)
    if _trace:
        kernel.last_results = res
    return out


# revision 9
# speedup vs baseline: 1.0888x; 1.0559x over previous
"""Multi-head causal attention kernel for Trainium2 (8 NeuronCores, batch-parallel).

Problem: B=8, Tx=Tz=1024, Dx=Dz=1024, Datt=Dmid=64, H=16, Dout=1024, causal mask.
Sharding: batch dim across the 8 cores (one batch element per core) - weights
replicated, no collectives needed.

Per-core dataflow (matmul operands in fp16, all accumulation in fp32 PSUM):
  xT,zT : host-transposed activations [d, t]
  V     = zT.T @ Wv_cat + bv    -> [z, h*65+e] with a ones column per head
                                   (bias via K=1 matmul; ones col via DVE fill)
  per head-pair P (2 heads at partition offsets 0/64):
    QT_P = Wq_cat[:,P].T @ xT + bq  -> [128(he), 1024(x)]   (evict DVE+bias)
    KT_P = Wk_cat[:,P].T @ zT + bk  -> [128(he), 1024(z)]
    S^T  = lhsT=KT[64,128] x rhs=QT[64,512] -> 2-bank psum [z, 2*x] (row-packed)
    A^T  = exp(S^T/8) both heads in one ACT op, causal-trimmed, diag masked (DVE)
    yT   = V_aug.T @ A^T -> psum [65, x]: rows 0..63 = y^T, row 64 = sumexp
    norm: 1/sumexp (DVE approx) -> partition_broadcast (GPSIMD) -> mul (DVE)
  out  = yT_cat.T @ Wp + bp  (psum -> SBUF via ACT -> DRAM)
"""
import sys

sys.path.insert(0, "/opt/trn_rl_repo")

import numpy as np

import concourse.bacc as bacc
import concourse.mybir as mybir
import concourse.tile as tile
from concourse.bass_utils import run_bass_kernel_spmd

F32 = mybir.dt.float32
FP16 = mybir.dt.float16

B, T, D, E, H = 8, 1024, 1024, 64, 16
NK = D // 128          # 8 contraction tiles
NP = H // 2            # 8 head pairs
NJ = T // 128          # 8 z tiles
NC = T // 512          # 2 x chunks
SCALE = 0.125          # 1/sqrt(64)


def build_program():
    nc = bacc.Bacc("TRN2", target_bir_lowering=False, debug=False)

    xT_d = nc.dram_tensor("xT", [D, T], FP16, kind="ExternalInput")
    zT_d = nc.dram_tensor("zT", [D, T], FP16, kind="ExternalInput")
    wq_d = nc.dram_tensor("wq", [D, H * E], FP16, kind="ExternalInput")
    wk_d = nc.dram_tensor("wk", [D, H * E], FP16, kind="ExternalInput")
    wv_d = nc.dram_tensor("wv", [D, H * E], FP16, kind="ExternalInput")
    wp_d = nc.dram_tensor("wp", [H * E, D], FP16, kind="ExternalInput")
    bqk_d = nc.dram_tensor("bqk", [128, 16], F32, kind="ExternalInput")
    bvp_d = nc.dram_tensor("bvp", [65, H * E], FP16, kind="ExternalInput")
    bvb_d = nc.dram_tensor("bvb", [128, H * E], FP16, kind="ExternalInput")
    bpb_d = nc.dram_tensor("bpb", [128, H * E], F32, kind="ExternalInput")
    maskt_d = nc.dram_tensor("maskt", [128, 256], FP16, kind="ExternalInput")
    ones128_d = nc.dram_tensor("ones128", [65, 128], FP16, kind="ExternalInput")
    out_d = nc.dram_tensor("out", [T, D], F32, kind="ExternalOutput")

    Exp = mybir.ActivationFunctionType.Exp

    with tile.TileContext(nc) as tc:
        with (
            tc.tile_pool(name="big", bufs=1) as big,
            tc.tile_pool(name="wf", bufs=2) as wf,
            tc.tile_pool(name="wb", bufs=4) as wb,
            tc.tile_pool(name="qk", bufs=4) as qk,
            tc.tile_pool(name="apool", bufs=8) as apool,
            tc.tile_pool(name="norm", bufs=3) as norm,
            tc.tile_pool(name="opool", bufs=3) as opool,
            tc.tile_pool(name="cst", bufs=1) as cst,
            tc.tile_pool(name="mps", bufs=2, space="PSUM") as mps,
            tc.tile_pool(name="sps", bufs=2, space="PSUM") as sps,
            tc.tile_pool(name="yps", bufs=2, space="PSUM") as yps,
        ):
            # ---- constants ----
            bqk_t = cst.tile([128, 16], F32)
            bvp_t = cst.tile([65, H * E], FP16)
            bvb_t = cst.tile([128, H * E], FP16)
            bpb_t = cst.tile([128, H * E], F32)
            maskt_t = cst.tile([128, 256], FP16)
            ones128_t = cst.tile([65, 128], FP16)
            onesf_t = cst.tile([128, 16], FP16)
            def _load_consts():
                nc.sync.dma_start(bqk_t[:], bqk_d.ap())
                nc.sync.dma_start(bvp_t[:], bvp_d.ap())
                nc.sync.dma_start(bvb_t[:], bvb_d.ap())
                nc.sync.dma_start(bpb_t[:], bpb_d.ap())
                nc.sync.dma_start(maskt_t[:], maskt_d.ap())
                nc.sync.dma_start(ones128_t[:], ones128_d.ap())
            nc.gpsimd.memset(onesf_t[:], 1.0)

            # ---- resident activations ----
            xT_t = big.tile([128, NK, T], FP16, tag="xT")
            zT_t = big.tile([128, NK, T], FP16, tag="zT")
            V_t = big.tile([128, NJ, H * 65], FP16, tag="V")
            yT_t = big.tile([128, NP, T], FP16, tag="yT")
            wv_r = wv_d.ap().rearrange("(k p) he -> p k he", p=128)
            wp_r = wp_d.ap().rearrange("(k p) dout -> p k dout", p=128)
            wq_r = wq_d.ap().rearrange("(k p) he -> p k he", p=128)
            wk_r = wk_d.ap().rearrange("(k p) he -> p k he", p=128)
            xT_r = xT_d.ap().rearrange("(k p) t -> p k t", p=128)
            zT_r = zT_d.ap().rearrange("(k p) t -> p k t", p=128)
            # zT + first wv half first: V-phase matmuls can start after k=0 lands
            wvh0 = wf.tile([128, NK, 512], FP16, tag="wf", name="wvh0")
            for k in range(NK):
                nc.sync.dma_start(zT_t[:, k, :], zT_r[:, k, :])
                nc.sync.dma_start(wvh0[:, k, :], wv_r[:, k, 0:512])
            _load_consts()
            for k in range(NK):
                nc.sync.dma_start(xT_t[:, k, :], xT_r[:, k, :])

            # ---- V phase: V[z, he] = zT.T @ Wv + bv (65-col/head layout) ----
            for vc in range(2):
                if vc == 0:
                    wvh = wvh0
                else:
                    wvh = wf.tile([128, NK, 512], FP16, tag="wf")
                    nc.sync.dma_start(wvh[:], wv_r[:, :, vc * 512:(vc + 1) * 512])
                for zb in range(NJ):
                    ps = mps.tile([128, 512], F32, tag="mps")
                    for k in range(NK):
                        nc.tensor.matmul(
                            ps[:], zT_t[:, k, zb * 128:(zb + 1) * 128], wvh[:, k, :],
                            start=(k == 0), stop=(k == NK - 1),
                        )
                    dst = V_t[:, zb, vc * 520:(vc + 1) * 520].rearrange(
                        "p (h c) -> p h c", c=65)[:, :, 0:64]
                    nc.vector.tensor_add(
                        dst, ps[:].rearrange("p (h c) -> p h c", c=64),
                        bvb_t[:, vc * 512:(vc + 1) * 512].rearrange("p (h c) -> p h c", c=64))
            for zb in range(NJ):
                ones_dst = V_t[:, zb, :].rearrange("p (h c) -> p h c", c=65)[:, :, 64:65]
                nc.vector.tensor_copy(ones_dst, onesf_t[:].rearrange("p (h c) -> p h c", c=1))

            # ---- prefetch both Wp halves (wf slots free up after V-phase use) ----
            wph = []
            for dc in range(2):
                w = wf.tile([128, NK, 512], FP16, tag="wf", name=f"wph{dc}")
                nc.sync.dma_start(w[:], wp_r[:, :, dc * 512:(dc + 1) * 512])
                wph.append(w)

            # ---- head-pair loop ----
            for P in range(NP):
                wqP = wb.tile([128, NK, 128], FP16, tag="wb")
                nc.sync.dma_start(wqP[:], wq_r[:, :, P * 128:(P + 1) * 128])
                wkP = wb.tile([128, NK, 128], FP16, tag="wb")
                nc.sync.dma_start(wkP[:], wk_r[:, :, P * 128:(P + 1) * 128])

                QT = qk.tile([128, T], FP16, tag="qk")
                for c in range(NC):
                    ps = mps.tile([128, 512], F32, tag="mps")
                    for k in range(NK):
                        nc.tensor.matmul(
                            ps[:], wqP[:, k, :], xT_t[:, k, c * 512:(c + 1) * 512],
                            start=(k == 0), stop=(k == NK - 1),
                        )
                    nc.vector.tensor_scalar_add(QT[:, c * 512:(c + 1) * 512], ps[:],
                                                bqk_t[:, P:P + 1])
                KT = qk.tile([128, T], FP16, tag="qk")
                for c in range(NC):
                    ps = mps.tile([128, 512], F32, tag="mps")
                    for k in range(NK):
                        nc.tensor.matmul(
                            ps[:], wkP[:, k, :], zT_t[:, k, c * 512:(c + 1) * 512],
                            start=(k == 0), stop=(k == NK - 1),
                        )
                    nc.vector.tensor_scalar_add(KT[:, c * 512:(c + 1) * 512], ps[:],
                                                bqk_t[:, 8 + P:9 + P])

                # attention for the two heads of this pair
                for c in range(NC):
                    jlive = [j for j in range(NJ) if 128 * j <= 512 * c + 511]
                    yp = [yps.tile([65, 512], F32, tag="yps", name=f"yp{P}_{c}_{h01}")
                          for h01 in range(2)]
                    for j in jlive:
                        kband = j - 4 * c
                        x0 = 128 * max(kband, 0)
                        sp = sps.tile([128, 1024], F32, tag="sps")
                        at = apool.tile([128, 1024], FP16, tag="at")
                        for h01 in range(2):
                            hoff = 64 * h01
                            nc.tensor.matmul(
                                sp[:, h01 * 512 + x0:(h01 + 1) * 512],
                                KT[hoff:hoff + 64, j * 128:(j + 1) * 128],
                                QT[hoff:hoff + 64, c * 512 + x0:(c + 1) * 512],
                                start=True, stop=True,
                            )
                        # one exp over both heads' regions (strided 2-bank AP)
                        sp_v = sp[:].rearrange("p (h x) -> p h x", x=512)[:, :, x0:512]
                        at_v = at[:].rearrange("p (h x) -> p h x", x=512)[:, :, x0:512]
                        nc.scalar.activation(at_v, sp_v, Exp, bias=0.0, scale=SCALE)
                        if kband >= 0:
                            at_m = at[:].rearrange(
                                "p (h x) -> p h x", x=512)[:, :, x0:x0 + 128]
                            mk_m = maskt_t[:].rearrange("p (h x) -> p h x", x=128)
                            nc.vector.tensor_mul(at_m, at_m, mk_m)
                        for h01 in range(2):
                            h = 2 * P + h01
                            nc.tensor.matmul(
                                yp[h01][:, x0:512],
                                V_t[:, j, h * 65:(h + 1) * 65],
                                at[:, h01 * 512 + x0:(h01 + 1) * 512],
                                start=(j == jlive[0]), stop=(j == jlive[-1]),
                                skip_group_check=True,
                            )
                    # normalization + eviction to packed pair layout
                    for h01 in range(2):
                        hoff = 64 * h01
                        se_t = norm.tile([1, 512], F32, tag="se")
                        nc.vector.tensor_copy(se_t[:], yp[h01][64:65, :])
                        r_t = norm.tile([1, 512], F32, tag="rt")
                        nc.vector.reciprocal_approx_fast(r_t[:], se_t[:])
                        bc_t = norm.tile([64, 512], F32, tag="bc")
                        nc.gpsimd.partition_broadcast(bc_t[:], r_t[:])
                        nc.vector.tensor_mul(
                            yT_t[hoff:hoff + 64, P, c * 512:(c + 1) * 512],
                            yp[h01][0:64, :], bc_t[:])

            # ---- output projection: out = yT_cat.T @ Wp + bp ----
            for dc in range(2):
                for m in range(NJ):
                    ps = mps.tile([128, 512], F32, tag="mps")
                    for ht in range(NP):
                        nc.tensor.matmul(
                            ps[:], yT_t[:, ht, m * 128:(m + 1) * 128], wph[dc][:, ht, :],
                            start=(ht == 0), stop=(ht == NP - 1),
                        )
                    o_t = opool.tile([128, 512], F32, tag="ot")
                    nc.vector.tensor_add(o_t[:], ps[:], bpb_t[:, dc * 512:(dc + 1) * 512])
                    nc.sync.dma_start(
                        out_d.ap()[m * 128:(m + 1) * 128, dc * 512:(dc + 1) * 512],
                        o_t[:])

    nc.compile()
    return nc


_CACHED_NC = None


def _get_program():
    global _CACHED_NC
    if _CACHED_NC is None:
        _CACHED_NC = build_program()
    return _CACHED_NC


def _ones65x128():
    a = np.zeros((65, 128), np.float16)
    a[0] = 1.0
    a[64] = 1.0
    return a


def _prep_shared(Wq, bq, Wk, bk, Wv, bv, Wp, bp, mask):
    assert np.array_equal(
        np.asarray(mask), np.tril(np.ones((T, T), dtype=bool))
    ), "kernel specialized for causal (tril) mask"
    wq = np.ascontiguousarray(
        np.asarray(Wq, np.float32).transpose(1, 0, 2).reshape(D, H * E).astype(np.float16))
    wk = np.ascontiguousarray(
        np.asarray(Wk, np.float32).transpose(1, 0, 2).reshape(D, H * E).astype(np.float16))
    wv = np.ascontiguousarray(
        np.asarray(Wv, np.float32).transpose(1, 0, 2).reshape(D, H * E).astype(np.float16))
    wp = np.ascontiguousarray(np.asarray(Wp, np.float32).astype(np.float16))
    bq_c = np.asarray(bq, np.float32).reshape(-1)
    bk_c = np.asarray(bk, np.float32).reshape(-1)
    bqk = np.concatenate(
        [bq_c.reshape(8, 128).T, bk_c.reshape(8, 128).T], axis=1
    ).astype(np.float32)
    bvp = np.zeros((65, H * E), np.float16)
    bvp[0] = np.asarray(bv, np.float32).reshape(-1).astype(np.float16)
    bvp[64] = np.asarray(bp, np.float32).reshape(-1).astype(np.float16)
    tri = np.triu(np.ones((128, 128), np.float16))  # allow z <= x
    maskt = np.concatenate([tri, tri], axis=1)      # [128, 256] for both heads
    bvb = np.ascontiguousarray(np.broadcast_to(
        np.asarray(bv, np.float32).reshape(1, -1), (128, H * E)).astype(np.float16))
    bpb = np.ascontiguousarray(np.broadcast_to(
        np.asarray(bp, np.float32).reshape(1, -1), (128, H * E)).astype(np.float32))
    return {
        "wq": wq, "wk": wk, "wv": wv, "wp": wp,
        "bqk": np.ascontiguousarray(bqk), "bvp": np.ascontiguousarray(bvp),
        "bvb": bvb, "bpb": bpb,
        "maskt": np.ascontiguousarray(maskt),
        "ones128": _ones65x128(),
    }


def kernel(x, z, Wq, bq, Wk, bk, Wv, bv, Wp, bp, mask, _trace=False, _trace_kwargs=None):
    x = np.asarray(x, np.float32)
    z = np.asarray(z, np.float32)
    shared = _prep_shared(Wq, bq, Wk, bk, Wv, bv, Wp, bp, mask)
    in_maps = []
    for b in range(B):
        m = dict(shared)
        m["xT"] = np.ascontiguousarray(x[b].T.astype(np.float16))
        m["zT"] = np.ascontiguousarray(z[b].T.astype(np.float16))
        in_maps.append(m)
    nc = _get_program()
    res = run_bass_kernel_spmd(
        nc, in_maps, core_ids=list(range(B)),
        trace=_trace, **(_trace_kwargs or {}),
    )
    out = np.stack([r["out"] for r in res.results]).astype(np.float32)
    if _trace:
        kernel.last_results = res
    return out
